# revision 1
# baseline (speedup 1.0000x reference)
"""Capsule-routing kernel for Trainium2, 8-core batch-parallel.

Reference computation (per example, In=4096, D=256, N=16, K=16, routings=3):
    u_hat = (x @ W).reshape(In, N, K)           # [In, 256] with m = n*16+k
    b = 0
    for j in range(3):
        c = softmax(b, axis=n)                   # [In, N]
        outputs = squash(sum_i c[i,n] u_hat[i,n,:])   # [N, K]
        if j < 2: b[i,n] = sum_k outputs[n,k] u_hat[i,n,k]

Device strategy per core (4 examples):
  - host supplies xT [2,128,4096] per example (d on partitions)
  - PE Form3: u_hat[i,m] f32 (stationary = xT 128x128 slices, rhs = W)
  - PE Form2: u_hatT[m,i] -> stored bf16 (only feeds b-update)
  - outputs-acc: 32 accumulating matmuls lhsT=c-tile [128,16], rhs=u_hat tile
  - b-update: 64 matmuls lhsT=u_hatT slice [128,128] bf16, rhs=S [128,16] bf16
  - softmax over n: exp (ScalarE) + segmented reduce + reciprocal (DVE)
"""

import sys
from contextlib import ExitStack

sys.path.insert(0, "/opt/trn_rl_repo")

import numpy as np

import concourse.bass as bass
import concourse.mybir as mybir
import concourse.tile as tile
from concourse import bacc
from concourse.bass_utils import run_bass_kernel_spmd

F32 = mybir.dt.float32
F32R = mybir.dt.float32r
BF16 = mybir.dt.bfloat16
U32 = mybir.dt.uint32

N_CORES = 8
B = 32
IN = 4096
D = 256
N = 16
K = 16
M = N * K  # 256
EPS = 1e-7


def build_kernel(n_ex=4, n_tiles=32, routings=3, ut_dtype="bf16", ut_bufs=2):
    """Build the per-core Bass module. In = n_tiles*128."""
    In = n_tiles * 128
    nc = bacc.Bacc("TRN2", target_bir_lowering=False, debug=False,
                   num_devices=N_CORES)

    # DRAM I/O
    xT_d = nc.dram_tensor("xT", [n_ex, 2, 128, In], F32R, kind="ExternalInput")
    Wt_d = nc.dram_tensor("Wt", [2, 128, M], F32R, kind="ExternalInput")
    ones16_d = nc.dram_tensor("ones16", [128, N], F32R, kind="ExternalInput")
    bmask_d = nc.dram_tensor("bmask", [N, M], F32, kind="ExternalInput")
    id16_d = nc.dram_tensor("id16", [N, N], F32, kind="ExternalInput")
    sel4_d = nc.dram_tensor("sel4", [128, N], F32R, kind="ExternalInput")
    out_d = nc.dram_tensor("out", [n_ex, N, K], F32, kind="ExternalOutput")

    with tile.TileContext(nc) as tc, ExitStack() as ctx:
        # ---- pools ----
        const_pool = ctx.enter_context(tc.tile_pool(name="consts", bufs=1))
        xT_pool = ctx.enter_context(tc.tile_pool(name="xT", bufs=2))
        u_pool = ctx.enter_context(tc.tile_pool(name="u", bufs=2))
        uT_pool = ctx.enter_context(tc.tile_pool(name="uT", bufs=ut_bufs))
        small_pool = ctx.enter_context(tc.tile_pool(name="small", bufs=4))
        out_pool = ctx.enter_context(tc.tile_pool(name="outstage", bufs=1))

        ps_u = ctx.enter_context(tc.tile_pool(name="ps_u", bufs=2, space="PSUM"))
        ps_uT = ctx.enter_context(tc.tile_pool(name="ps_uT", bufs=1, space="PSUM"))
        ps_acc = ctx.enter_context(tc.tile_pool(name="ps_acc", bufs=2, space="PSUM"))
        ps_b = ctx.enter_context(tc.tile_pool(name="ps_b", bufs=2, space="PSUM"))
        ps_s = ctx.enter_context(tc.tile_pool(name="ps_s", bufs=1, space="PSUM"))

        # ---- constants ----
        Wt = const_pool.tile([128, 2, M], F32R, tag="Wt")
        nc.sync.dma_start(Wt[:], Wt_d.ap().rearrange("c p m -> p c m"))
        ones16 = const_pool.tile([128, N], F32R, tag="ones16")
        nc.sync.dma_start(ones16[:], ones16_d[:])
        bmask = const_pool.tile([N, M], F32, tag="bmask")
        nc.sync.dma_start(bmask[:], bmask_d[:])
        id16 = const_pool.tile([N, N], F32, tag="id16")
        nc.sync.dma_start(id16[:], id16_d[:])
        sel4 = const_pool.tile([128, N], F32R, tag="sel4")
        nc.sync.dma_start(sel4[:], sel4_d[:])

        out_stage = out_pool.tile([N, n_ex * K], F32, tag="outst")
        eps_t = const_pool.tile([N, 1], F32, tag="eps")
        nc.vector.memset(eps_t[:], EPS)

        for e in range(n_ex):
            # ======== load xT ========
            xT = xT_pool.tile([128, 2, In], F32R, tag="xT")
            nc.sync.dma_start(xT[:, 0, :], xT_d[e, 0])
            nc.sync.dma_start(xT[:, 1, :], xT_d[e, 1])

            # ======== Form 3: u_hat[i, m], tiles of [128, 256] ========
            u_sb = u_pool.tile([128, n_tiles, M], F32R, tag="u")
            for tp in range(n_tiles // 2):
                pu = ps_u.tile([128, 2, M], F32, tag="ps_u")
                for half in range(2):
                    t = 2 * tp + half
                    for dc in range(2):
                        nc.tensor.matmul(
                            pu[:, half, :],
                            xT[:, dc, 128 * t:128 * (t + 1)],
                            Wt[:, dc, :],
                            start=(dc == 0), stop=(dc == 1),
                            skip_group_check=True)
                if tp % 2 == 0:
                    nc.scalar.copy(u_sb[:, 2 * tp:2 * tp + 2, :], pu[:])
                else:
                    nc.vector.tensor_copy(u_sb[:, 2 * tp:2 * tp + 2, :], pu[:])

            # ======== Form 2: u_hatT[m, i] in bf16 ========
            uT_dt = {"bf16": BF16, "f32": F32, "f32r": F32R}[ut_dtype]
            uT_sb = uT_pool.tile([128, 2, In], uT_dt, tag="uT")
            n_ch = In // 512
            for mt in range(2):
                for ch in range(n_ch):
                    puT = ps_uT.tile([128, 512], F32, tag="ps_uT")
                    for dc in range(2):
                        nc.tensor.matmul(
                            puT[:],
                            Wt[:, dc, 128 * mt:128 * (mt + 1)],
                            xT[:, dc, 512 * ch:512 * (ch + 1)],
                            start=(dc == 0), stop=(dc == 1))
                    if ch % 2 == 0:
                        nc.scalar.copy(uT_sb[:, mt, 512 * ch:512 * (ch + 1)],
                                       puT[:])
                    else:
                        nc.vector.tensor_copy(
                            uT_sb[:, mt, 512 * ch:512 * (ch + 1)], puT[:])

            # ======== routing ========
            c_all = small_pool.tile([128, n_tiles, N], F32R, tag="c_all")
            for j in range(routings):
                # --- outputs accumulation -> acc_ps [16, 256] ---
                acc_ps = ps_acc.tile([N, M], F32, tag="acc")
                for t in range(n_tiles):
                    lhsT = ones16[:] if j == 0 else c_all[:, t, :]
                    nc.tensor.matmul(acc_ps[:], lhsT, u_sb[:, t, :],
                                     start=(t == 0), stop=(t == n_tiles - 1))

                # --- squash ---
                o_full = small_pool.tile([N, M], F32, tag="o_full")
                nc.scalar.copy(o_full[:], acc_ps[:])
                om = small_pool.tile([N, M], F32, tag="om")
                nrm2 = small_pool.tile([N, 1], F32, tag="nrm2")
                sq = small_pool.tile([N, M], F32, tag="sq")
                nc.vector.tensor_mul(om[:], o_full[:], bmask[:])
                nc.scalar.activation(sq[:], om[:],
                                     mybir.ActivationFunctionType.Square,
                                     accum_out=nrm2[:])
                # rinv = 1/sqrt(nrm2 + eps) via bit-trick + 2 Newton steps
                # (keeps ScalarE on the exp_and_friends ACT table: no
                # table-reload thrash from Sqrt)
                A = mybir.AluOpType
                xe = small_pool.tile([N, 1], F32, tag="xe")
                nc.vector.tensor_scalar_add(xe[:], nrm2[:], EPS)
                sbits = small_pool.tile([N, 1], U32, tag="sbits")
                nc.vector.tensor_scalar(sbits[:], xe[:].bitcast(U32), 1, None,
                                        op0=A.logical_shift_right)
                ybits = small_pool.tile([N, 1], U32, tag="ybits")
                nc.vector.tensor_scalar(ybits[:], sbits[:], -1.0,
                                        float(0x5F3759DF),
                                        op0=A.mult, op1=A.add)
                y = ybits[:].bitcast(F32)
                t1 = small_pool.tile([N, 1], F32, tag="t1")
                t2 = small_pool.tile([N, 1], F32, tag="t2")
                rinv = small_pool.tile([N, 1], F32, tag="rinv")
                n_newton = 2 if j == routings - 1 else 1
                for it in range(n_newton):
                    nc.vector.tensor_mul(t1[:], xe[:], y)
                    nc.vector.tensor_mul(t2[:], t1[:], y)
                    nc.vector.tensor_scalar(t2[:], t2[:], -0.5, 1.5,
                                            op0=A.mult, op1=A.add)
                    dst = rinv if it == n_newton - 1 else small_pool.tile(
                        [N, 1], F32, tag="ynext")
                    nc.vector.tensor_mul(dst[:], t2[:], y)
                    y = dst[:]
                o_n = small_pool.tile([N, M], F32, tag="o_n")
                nc.vector.tensor_scalar_mul(o_n[:], om[:], rinv[:])

                if j == routings - 1:
                    # final extraction: out[n,k] = sum_g o_n[n, g*16+k]
                    nc.vector.tensor_reduce(
                        out_stage[:, K * e:K * (e + 1)],
                        o_n[:].rearrange("p (g k) -> p k g", k=K),
                        axis=mybir.AxisListType.X, op=mybir.AluOpType.add)
                    continue

                # --- S build: S[mc] = transpose(o_n[:, mc*128:...]) bf16 ---
                S = small_pool.tile([128, 2, N], uT_dt, tag="S")
                sps = ps_s.tile([128, 2, N], F32, tag="s_ps")
                for mc in range(2):
                    nc.tensor.transpose(sps[:, mc, :],
                                        o_n[:, 128 * mc:128 * (mc + 1)],
                                        id16[:])
                nc.scalar.copy(S[:], sps[:])

                # --- b update: b[i, (t,n)] = sum_m u_hat[i,m] S[m,n] ---
                b_ps = ps_b.tile([128, n_tiles * N], F32, tag="b_ps")
                for t in range(n_tiles):
                    for mc in range(2):
                        nc.tensor.matmul(
                            b_ps[:, N * t:N * (t + 1)],
                            uT_sb[:, mc, 128 * t:128 * (t + 1)],
                            S[:, mc, :],
                            start=(mc == 0), stop=(mc == 1),
                            skip_group_check=True)
                b_all = small_pool.tile([128, n_tiles, N], F32, tag="b_all")
                nc.scalar.copy(
                    b_all[:], b_ps[:].rearrange("p (t n) -> p t n", n=N))

                # --- softmax over n ---
                e_all = small_pool.tile([128, n_tiles, N], F32, tag="e_all")
                nc.scalar.activation(e_all[:], b_all[:],
                                     mybir.ActivationFunctionType.Exp)
                s_sum = small_pool.tile([128, n_tiles], F32, tag="s_sum")
                nc.vector.tensor_reduce(s_sum[:], e_all[:],
                                        axis=mybir.AxisListType.X,
                                        op=mybir.AluOpType.add)
                s_r = small_pool.tile([128, n_tiles], F32, tag="s_r")
                nc.vector.reciprocal(s_r[:], s_sum[:])
                nc.vector.tensor_mul(
                    c_all[:], e_all[:],
                    s_r[:].to_broadcast([128, n_tiles, N]))

        # ======== store outputs ========
        nc.sync.dma_start(out_d.ap().rearrange("e n k -> n e k"),
                          out_stage[:].rearrange("p (e k) -> p e k", k=K))

    nc.compile()
    return nc


_NC_CACHE = {}


def _get_nc(n_ex=4, n_tiles=32, routings=3, ut_dtype="f32r", ut_bufs=1):
    key = (n_ex, n_tiles, routings, ut_dtype, ut_bufs)
    if key not in _NC_CACHE:
        _NC_CACHE[key] = build_kernel(*key)
    return _NC_CACHE[key]


def make_const_inputs():
    ones16 = np.full((128, N), 1.0 / N, dtype=np.float32)
    bmask = np.zeros((N, M), dtype=np.float32)
    for n in range(N):
        bmask[n, n * K:(n + 1) * K] = 1.0
    id16 = np.eye(N, dtype=np.float32)
    sel4 = np.zeros((128, N), dtype=np.float32)
    for p in range(128):
        if p % 32 < N:
            sel4[p, p % 32] = 1.0
    return ones16, bmask, id16, sel4


def kernel(x, W, num_capsule=None, dim_capsule=None, routings=None, **_):
    x = np.asarray(x, dtype=np.float32)
    W = np.asarray(W, dtype=np.float32)
    assert x.shape == (B, IN, D), x.shape

    nc = _get_nc()
    ones16, bmask, id16, sel4 = make_const_inputs()
    Wt = np.ascontiguousarray(W[0].reshape(2, 128, M))

    n_per = B // N_CORES
    in_maps = []
    for c in range(N_CORES):
        xs = x[c * n_per:(c + 1) * n_per]              # [4, 4096, 256]
        xT = np.ascontiguousarray(
            xs.transpose(0, 2, 1)).reshape(n_per, 2, 128, IN)
        in_maps.append({"xT": xT, "Wt": Wt, "ones16": ones16,
                        "bmask": bmask, "id16": id16, "sel4": sel4})

    res = run_bass_kernel_spmd(nc, in_maps, core_ids=list(range(N_CORES)))
    out = np.concatenate([r["out"] for r in res.results], axis=0)
    return out.astype(np.float32)



# revision 7
# speedup vs baseline: 1.8985x; 1.8985x over previous
"""Capsule-routing kernel for Trainium2, 8-core batch-parallel (v2).

Reference computation (per example, In=4096, D=256, N=16, K=16, routings=3):
    u_hat = (x @ W).reshape(In, N, K)            # m = n*16+k
    b = 0
    for j in range(3):
        c = softmax(b, axis=n)                   # [In, N]
        outputs = squash(sum_i c[i,n] u_hat[i,n,:])   # [N, K]
        if j < 2: b[i,n] = sum_k outputs[n,k] u_hat[i,n,k]

Key algebraic restructure: u_hat is NEVER materialized.
    acc[n,m]  = sum_i c[i,n] u_hat[i,m]  = W^T CX where CX[d,n] = sum_i x[i,d] c[i,n]
    b[i,n]    = sum_m u_hat[i,m] S[m,n]  = x @ (W S) = x @ WS
so the only large tensors on device are x in two layouts (i-major for CX,
d-major for b), shipped from host as bf16. All PE outputs are tiny
([128,2,16] / [128,32,16]), eliminating the PSUM->SBUF copy traffic that
dominated the u_hat formulation. The kernel is DMA-bound (16 MiB/core).

squash rinv = (nrm2+eps)^-1/2 computed as exp(-0.5*ln(nrm2+eps)): Ln and Exp
share one activation table (natural_log_exp_and_others) -> no table reloads.
"""

import sys
from contextlib import ExitStack

sys.path.insert(0, "/opt/trn_rl_repo")

import ml_dtypes
import numpy as np

import concourse.bass as bass
import concourse.mybir as mybir
import concourse.tile as tile
from concourse import bacc
from concourse.bass_utils import run_bass_kernel_spmd

F32 = mybir.dt.float32
F32R = mybir.dt.float32r
BF16 = mybir.dt.bfloat16

N_CORES = 8
B = 32
IN = 4096
D = 256
N = 16
K = 16
M = N * K  # 256
T = IN // 128  # 32 tiles
EPS = 1e-7
BF = ml_dtypes.bfloat16

Act = mybir.ActivationFunctionType
Axis = mybir.AxisListType
Alu = mybir.AluOpType


def build_kernel(n_ex=4, routings=3):
    nc = bacc.Bacc("TRN2", target_bir_lowering=False, debug=False,
                   num_devices=N_CORES)

    # ---- DRAM I/O ----
    xq_d = nc.dram_tensor("xq", [n_ex, 128, T, D], BF16, kind="ExternalInput")
    xTq_d = nc.dram_tensor("xTq", [n_ex, 128, 2, IN], BF16,
                           kind="ExternalInput")
    Wt_d = nc.dram_tensor("Wt", [128, 2, M], F32R, kind="ExternalInput")
    WTt_d = nc.dram_tensor("WTt", [128, 2, D], F32R, kind="ExternalInput")
    mask_d = nc.dram_tensor("maskmn", [128, 2, N], BF16, kind="ExternalInput")
    ones16_d = nc.dram_tensor("ones16", [128, N], BF16, kind="ExternalInput")
    onesc_d = nc.dram_tensor("onesc", [128, 1], F32R, kind="ExternalInput")
    bmask_d = nc.dram_tensor("bmask", [N, M], F32, kind="ExternalInput")
    out_d = nc.dram_tensor("out", [n_ex, N, K], F32, kind="ExternalOutput")

    with tile.TileContext(nc) as tc, ExitStack() as ctx:
        # ---- pools ----
        const_pool = ctx.enter_context(tc.tile_pool(name="consts", bufs=1))
        x_pool = ctx.enter_context(tc.tile_pool(name="x", bufs=1))
        xT_pool = ctx.enter_context(tc.tile_pool(name="xT", bufs=1))
        sm_pool = ctx.enter_context(tc.tile_pool(name="sm", bufs=4))
        ec_pool = ctx.enter_context(tc.tile_pool(name="ec", bufs=2))
        out_pool = ctx.enter_context(tc.tile_pool(name="outst", bufs=1))

        # one bank-sized tile holds all small matmul outputs of an iteration:
        # slot 0 = CX^T, slot 1 = accT, slot 2 = WS, slot 3 = nrm row
        ps_sm = ctx.enter_context(tc.tile_pool(name="ps_sm", bufs=3,
                                               space="PSUM"))
        ps_b = ctx.enter_context(tc.tile_pool(name="ps_b", bufs=2,
                                              space="PSUM"))
        ps_acc = ctx.enter_context(tc.tile_pool(name="ps_acc", bufs=1,
                                                space="PSUM"))

        # ---- constants ----
        Wt = const_pool.tile([128, 2, M], F32R, tag="Wt")
        nc.sync.dma_start(Wt[:], Wt_d[:])
        WTt = const_pool.tile([128, 2, D], F32R, tag="WTt")
        nc.sync.dma_start(WTt[:], WTt_d[:])
        mask = const_pool.tile([128, 2, N], BF16, tag="mask")
        nc.sync.dma_start(mask[:], mask_d[:])
        ones16 = const_pool.tile([128, N], BF16, tag="ones16")
        nc.sync.dma_start(ones16[:], ones16_d[:])
        onesc = const_pool.tile([128, 1], F32R, tag="onesc")
        nc.sync.dma_start(onesc[:], onesc_d[:])
        bmask = const_pool.tile([N, M], F32, tag="bmask")
        nc.sync.dma_start(bmask[:], bmask_d[:])

        eps_row = const_pool.tile([1, 1], F32, tag="eps_row")
        nc.vector.memset(eps_row[:], EPS)
        eps16 = const_pool.tile([N, 1], F32, tag="eps16")
        nc.vector.memset(eps16[:], EPS)
        out_stage = out_pool.tile([N, n_ex * K], F32, tag="outst")

        # ---- input DMAs (all up front; transfers serialize on the DMA bus,
        # compute for example e streams in behind its own arrivals) ----
        x_sb, xT_sb = [], []
        for e in range(n_ex):
            xs = x_pool.tile([128, T, D], BF16, tag=f"x{e}")
            nc.sync.dma_start(xs[:], xq_d[e])
            x_sb.append(xs)
            xt = xT_pool.tile([128, 2, IN], BF16, tag=f"xT{e}")
            nc.sync.dma_start(xt[:], xTq_d[e])
            xT_sb.append(xt)

        # ---- routing ----
        for e in range(n_ex):
            c_sb = None
            for j in range(routings):
                last = j == routings - 1
                # --- CX^T[d, n] = sum_i x[i,d] c[i,n] ---
                psm = ps_sm.tile([128, 4, 2, N], F32, tag="sm")
                pcx = psm[:, 0]
                rhs_c = ones16 if j == 0 else c_sb
                for dc in range(2):
                    for t in range(T):
                        rhs = rhs_c[:] if j == 0 else rhs_c[:, t, :]
                        nc.tensor.matmul(
                            pcx[:, dc, :],
                            x_sb[e][:, t, 128 * dc:128 * (dc + 1)],
                            rhs,
                            start=(t == 0), stop=(t == T - 1),
                            skip_group_check=True)
                cx = sm_pool.tile([128, 2, N], F32R, tag="cx_sb")
                nc.scalar.copy(cx[:], pcx[:])

                if last:
                    # --- final: acc[n,m] = (W^T CX)^T = CX^T... rows n ---
                    pac = ps_acc.tile([N, M], F32, tag="acc")
                    for dc in range(2):
                        nc.tensor.matmul(pac[:], cx[:, dc, :], Wt[:, dc, :],
                                         start=(dc == 0), stop=(dc == 1))
                    om = sm_pool.tile([N, M], F32, tag="om")
                    nc.vector.tensor_mul(om[:], pac[:], bmask[:])
                    sq = sm_pool.tile([N, M], F32, tag="sq")
                    nc.vector.tensor_mul(sq[:], om[:], om[:])
                    nrm2 = sm_pool.tile([N, 1], F32, tag="nrm2")
                    nc.vector.tensor_reduce(nrm2[:], sq[:], axis=Axis.X,
                                            op=Alu.add)
                    lnr = sm_pool.tile([N, 1], F32, tag="lnr")
                    nc.scalar.activation(lnr[:], nrm2[:], Act.Ln, bias=eps16[:])
                    rinv = sm_pool.tile([N, 1], F32, tag="rinv")
                    nc.scalar.activation(rinv[:], lnr[:], Act.Exp, scale=-0.5)
                    red = sm_pool.tile([N, K], F32, tag="red")
                    nc.vector.tensor_reduce(
                        red[:], om[:].rearrange("p (g k) -> p k g", k=K),
                        axis=Axis.X, op=Alu.add)
                    nc.vector.tensor_scalar_mul(
                        out_stage[:, K * e:K * (e + 1)], red[:], rinv[:])
                    continue

                # --- accT[m, n] = sum_d W[d,m] CX^T[d,n] ; S = mask * accT ---
                pat = psm[:, 1]
                for mc in range(2):
                    for dc in range(2):
                        nc.tensor.matmul(
                            pat[:, mc, :],
                            Wt[:, dc, 128 * mc:128 * (mc + 1)],
                            cx[:, dc, :],
                            start=(dc == 0), stop=(dc == 1),
                            skip_group_check=True)
                S = sm_pool.tile([128, 2, N], F32R, tag="S")
                nc.vector.tensor_mul(S[:], pat[:], mask[:])
                S2 = sm_pool.tile([128, 2, N], F32R, tag="S2")
                nc.vector.tensor_mul(S2[:], S[:], S[:])
                # nrm2 row [1, n] = sum_m S[m,n]^2
                pn = psm[0:1, 3, 0]
                for mc in range(2):
                    nc.tensor.matmul(pn[:], onesc[:], S2[:, mc, :],
                                     start=(mc == 0), stop=(mc == 1))
                ln_row = sm_pool.tile([1, N], F32, tag="ln_row")
                nc.scalar.activation(ln_row[:], pn[:], Act.Ln, bias=eps_row[:])
                r_row = sm_pool.tile([1, N], F32, tag="r_row")
                nc.scalar.activation(r_row[:], ln_row[:], Act.Exp, scale=-0.5)
                rb = sm_pool.tile([128, N], F32, tag="rb")
                nc.gpsimd.partition_broadcast(rb[:], r_row[:])
                # --- WS[d, n] = sum_m W[d,m] S[m,n], scaled by rinv[n] ---
                pws = psm[:, 2]
                for dc in range(2):
                    for mc in range(2):
                        nc.tensor.matmul(
                            pws[:, dc, :],
                            WTt[:, mc, 128 * dc:128 * (dc + 1)],
                            S[:, mc, :],
                            start=(mc == 0), stop=(mc == 1),
                            skip_group_check=True)
                ws = sm_pool.tile([128, 2, N], BF16, tag="ws_sb")
                for dc in range(2):
                    nc.vector.tensor_mul(ws[:, dc, :], pws[:, dc, :], rb[:])
                # --- b[i, (t,n)] = sum_d x[i,d] WS[d,n] ---
                pb = ps_b.tile([128, T, N], F32, tag="b")
                for t in range(T):
                    for dc in range(2):
                        nc.tensor.matmul(
                            pb[:, t, :],
                            xT_sb[e][:, dc, 128 * t:128 * (t + 1)],
                            ws[:, dc, :],
                            start=(dc == 0), stop=(dc == 1),
                            skip_group_check=True)
                # --- softmax over n ---
                e_sb = ec_pool.tile([128, T, N], BF16, tag="e")
                nc.scalar.activation(e_sb[:], pb[:], Act.Exp)
                s_sum = sm_pool.tile([128, T], F32, tag="s_sum")
                nc.vector.tensor_reduce(s_sum[:], e_sb[:], axis=Axis.X,
                                        op=Alu.add)
                s_r = sm_pool.tile([128, T], F32, tag="s_r")
                nc.vector.reciprocal(s_r[:], s_sum[:])
                c_sb = ec_pool.tile([128, T, N], BF16, tag="c")
                nc.vector.tensor_mul(c_sb[:], e_sb[:],
                                     s_r[:].to_broadcast([128, T, N]))

        # ---- store outputs ----
        nc.sync.dma_start(out_d.ap().rearrange("e n k -> n e k"),
                          out_stage[:].rearrange("p (e k) -> p e k", k=K))

    nc.compile()
    return nc


_NC_CACHE = {}


def _get_nc(n_ex=4, routings=3):
    key = (n_ex, routings)
    if key not in _NC_CACHE:
        _NC_CACHE[key] = build_kernel(*key)
    return _NC_CACHE[key]


def make_const_inputs(W):
    W0 = np.asarray(W[0], dtype=np.float32)  # [D, M]
    Wt = np.ascontiguousarray(
        W0.reshape(2, 128, M).transpose(1, 0, 2))        # [128, 2, M]
    WTt = np.ascontiguousarray(
        W0.T.reshape(2, 128, D).transpose(1, 0, 2))      # [128, 2, D]
    mask = np.zeros((128, 2, N), dtype=BF)
    for p in range(128):
        for mc in range(2):
            mask[p, mc, (mc * 128 + p) // K] = 1.0
    ones16 = np.full((128, N), 1.0 / N, dtype=BF)
    onesc = np.ones((128, 1), dtype=np.float32)
    bmask = np.zeros((N, M), dtype=np.float32)
    for n in range(N):
        bmask[n, n * K:(n + 1) * K] = 1.0
    return Wt, WTt, mask, ones16, onesc, bmask


def kernel(x, W, num_capsule=None, dim_capsule=None, routings=None, **_):
    x = np.asarray(x, dtype=np.float32)
    W = np.asarray(W, dtype=np.float32)
    assert x.shape == (B, IN, D), x.shape

    nc = _get_nc()
    Wt, WTt, mask, ones16, onesc, bmask = make_const_inputs(W)

    n_per = B // N_CORES
    in_maps = []
    for c in range(N_CORES):
        xs = x[c * n_per:(c + 1) * n_per]                 # [4, 4096, 256]
        xq = np.ascontiguousarray(
            xs.reshape(n_per, T, 128, D).transpose(0, 2, 1, 3)).astype(BF)
        xTq = np.ascontiguousarray(
            xs.transpose(0, 2, 1).reshape(n_per, 2, 128, IN)
            .transpose(0, 2, 1, 3)).astype(BF)
        in_maps.append({"xq": xq, "xTq": xTq, "Wt": Wt, "WTt": WTt,
                        "maskmn": mask, "ones16": ones16, "onesc": onesc,
                        "bmask": bmask})

    res = run_bass_kernel_spmd(nc, in_maps, core_ids=list(range(N_CORES)))
    out = np.concatenate([r["out"] for r in res.results], axis=0)
    return out.astype(np.float32)


# revision 12
# speedup vs baseline: 2.3124x; 1.2181x over previous
"""Capsule-routing kernel for Trainium2, 8-core batch-parallel (v2).

Reference computation (per example, In=4096, D=256, N=16, K=16, routings=3):
    u_hat = (x @ W).reshape(In, N, K)            # m = n*16+k
    b = 0
    for j in range(3):
        c = softmax(b, axis=n)                   # [In, N]
        outputs = squash(sum_i c[i,n] u_hat[i,n,:])   # [N, K]
        if j < 2: b[i,n] = sum_k outputs[n,k] u_hat[i,n,k]

Key algebraic restructure: u_hat is NEVER materialized.
    acc[n,m]  = sum_i c[i,n] u_hat[i,m]  = W^T CX where CX[d,n] = sum_i x[i,d] c[i,n]
    b[i,n]    = sum_m u_hat[i,m] S[m,n]  = x @ (W S) = x @ WS
so the only large tensors on device are x in two layouts (i-major for CX,
d-major for b), shipped from host as bf16. All PE outputs are tiny
([128,2,16] / [128,32,16]), eliminating the PSUM->SBUF copy traffic that
dominated the u_hat formulation. The kernel is DMA-bound (16 MiB/core).

squash rinv = (nrm2+eps)^-1/2 computed as exp(-0.5*ln(nrm2+eps)): Ln and Exp
share one activation table (natural_log_exp_and_others) -> no table reloads.
"""

import sys
from contextlib import ExitStack

sys.path.insert(0, "/opt/trn_rl_repo")

import ml_dtypes
import numpy as np

import concourse.bass as bass
import concourse.mybir as mybir
import concourse.tile as tile
from concourse import bacc
from concourse.bass_utils import run_bass_kernel_spmd

F32 = mybir.dt.float32
F32R = mybir.dt.float32r
BF16 = mybir.dt.bfloat16
U32 = mybir.dt.uint32

N_CORES = 8
B = 32
IN = 4096
D = 256
N = 16
K = 16
M = N * K  # 256
T = IN // 128  # 32 tiles
EPS = 1e-7
BF = ml_dtypes.bfloat16

Act = mybir.ActivationFunctionType
Axis = mybir.AxisListType
Alu = mybir.AluOpType


def emit_rsqrt(nc, pool, dst, src, shape, tagp, n_newton=1):
    """dst = (src + EPS)^-1/2 on DVE via bit-trick + Newton steps.

    Keeps ScalarE on the exp-only ACT table (a Ln/Exp-based rsqrt forces a
    1283ns table reload around every softmax Exp)."""
    A = Alu
    xe = pool.tile(shape, F32, tag=tagp + "xe")
    nc.vector.tensor_scalar_add(xe[:], src, EPS)
    sbits = pool.tile(shape, U32, tag=tagp + "sb")
    nc.vector.tensor_scalar(sbits[:], xe[:].bitcast(U32), 1, None,
                            op0=A.logical_shift_right)
    ybits = pool.tile(shape, U32, tag=tagp + "yb")
    nc.vector.tensor_scalar(ybits[:], sbits[:], -1.0, float(0x5F3759DF),
                            op0=A.mult, op1=A.add)
    y = ybits[:].bitcast(F32)
    t1 = pool.tile(shape, F32, tag=tagp + "t1")
    t2 = pool.tile(shape, F32, tag=tagp + "t2")
    for it in range(n_newton):
        nc.vector.tensor_mul(t1[:], xe[:], y)
        nc.vector.tensor_mul(t2[:], t1[:], y)
        nc.vector.tensor_scalar(t2[:], t2[:], -0.5, 1.5, op0=A.mult, op1=A.add)
        d = dst if it == n_newton - 1 else pool.tile(shape, F32,
                                                    tag=tagp + f"y{it}")
        nc.vector.tensor_mul(d[:], t2[:], y)
        y = d[:]


def build_kernel(n_ex=4, routings=3):
    nc = bacc.Bacc("TRN2", target_bir_lowering=False, debug=False,
                   num_devices=N_CORES)

    # ---- DRAM I/O ----
    xq_d = nc.dram_tensor("xq", [n_ex, 128, T, D], BF16, kind="ExternalInput")
    xTq_d = nc.dram_tensor("xTq", [n_ex, 128, 2, IN], BF16,
                           kind="ExternalInput")
    Wt_d = nc.dram_tensor("Wt", [128, 2, M], F32R, kind="ExternalInput")
    WTt_d = nc.dram_tensor("WTt", [128, 2, D], F32R, kind="ExternalInput")
    mask_d = nc.dram_tensor("maskmn", [128, 2, N], BF16, kind="ExternalInput")
    ones16_d = nc.dram_tensor("ones16", [128, N], BF16, kind="ExternalInput")
    onesc_d = nc.dram_tensor("onesc", [128, 1], F32R, kind="ExternalInput")
    bmask_d = nc.dram_tensor("bmask", [N, M], F32, kind="ExternalInput")
    out_d = nc.dram_tensor("out", [n_ex, N, K], F32, kind="ExternalOutput")

    with tile.TileContext(nc) as tc, ExitStack() as ctx:
        # ---- pools ----
        const_pool = ctx.enter_context(tc.tile_pool(name="consts", bufs=1))
        x_pool = ctx.enter_context(tc.tile_pool(name="x", bufs=1))
        xT_pool = ctx.enter_context(tc.tile_pool(name="xT", bufs=1))
        sm_pool = ctx.enter_context(tc.tile_pool(name="sm", bufs=4))
        ec_pool = ctx.enter_context(tc.tile_pool(name="ec", bufs=2))
        out_pool = ctx.enter_context(tc.tile_pool(name="outst", bufs=1))

        # one bank-sized tile holds all small matmul outputs of an iteration:
        # slot 0 = CX^T, slot 1 = accT, slot 2 = WS, slot 3 = nrm row
        ps_sm = ctx.enter_context(tc.tile_pool(name="ps_sm", bufs=3,
                                               space="PSUM"))
        ps_b = ctx.enter_context(tc.tile_pool(name="ps_b", bufs=2,
                                              space="PSUM"))
        ps_acc = ctx.enter_context(tc.tile_pool(name="ps_acc", bufs=1,
                                                space="PSUM"))

        # ---- constants ----
        Wt = const_pool.tile([128, 2, M], F32R, tag="Wt")
        nc.sync.dma_start(Wt[:], Wt_d[:])
        WTt = const_pool.tile([128, 2, D], F32R, tag="WTt")
        nc.sync.dma_start(WTt[:], WTt_d[:])
        mask = const_pool.tile([128, 2, N], BF16, tag="mask")
        nc.sync.dma_start(mask[:], mask_d[:])
        ones16 = const_pool.tile([128, N], BF16, tag="ones16")
        nc.sync.dma_start(ones16[:], ones16_d[:])
        onesc = const_pool.tile([128, 1], F32R, tag="onesc")
        nc.sync.dma_start(onesc[:], onesc_d[:])
        bmask = const_pool.tile([N, M], F32, tag="bmask")
        nc.sync.dma_start(bmask[:], bmask_d[:])

        out_stage = out_pool.tile([N, n_ex * K], F32, tag="outst")

        # ---- input DMAs (all up front; transfers serialize on the DMA bus,
        # compute for example e streams in behind its own arrivals) ----
        x_sb, xT_sb = [], []
        for e in range(n_ex):
            xs = x_pool.tile([128, T, D], BF16, tag=f"x{e}")
            nc.sync.dma_start(xs[:], xq_d[e])
            x_sb.append(xs)
            xt = xT_pool.tile([128, 2, IN], BF16, tag=f"xT{e}")
            nc.sync.dma_start(xt[:], xTq_d[e])
            xT_sb.append(xt)

        # ---- routing ----
        for e in range(n_ex):
            c_sb = None
            for j in range(routings):
                last = j == routings - 1
                # --- CX^T[d, n] = sum_i x[i,d] c[i,n] ---
                psm = ps_sm.tile([128, 4, 2, N], F32, tag="sm")
                pcx = psm[:, 0]
                rhs_c = ones16 if j == 0 else c_sb
                for dc in range(2):
                    for t in range(T):
                        rhs = rhs_c[:] if j == 0 else rhs_c[:, t, :]
                        nc.tensor.matmul(
                            pcx[:, dc, :],
                            x_sb[e][:, t, 128 * dc:128 * (dc + 1)],
                            rhs,
                            start=(t == 0), stop=(t == T - 1),
                            skip_group_check=True)
                cx = sm_pool.tile([128, 2, N], F32R, tag="cx_sb")
                nc.scalar.copy(cx[:], pcx[:])

                if last:
                    # --- final: acc[n,m] = (W^T CX)^T = CX^T... rows n ---
                    pac = ps_acc.tile([N, M], F32, tag="acc")
                    for dc in range(2):
                        nc.tensor.matmul(pac[:], cx[:, dc, :], Wt[:, dc, :],
                                         start=(dc == 0), stop=(dc == 1))
                    om = sm_pool.tile([N, M], F32, tag="om")
                    nc.vector.tensor_mul(om[:], pac[:], bmask[:])
                    sq = sm_pool.tile([N, M], F32, tag="sq")
                    nc.vector.tensor_mul(sq[:], om[:], om[:])
                    nrm2 = sm_pool.tile([N, 1], F32, tag="nrm2")
                    nc.vector.tensor_reduce(nrm2[:], sq[:], axis=Axis.X,
                                            op=Alu.add)
                    rinv = sm_pool.tile([N, 1], F32, tag="rinv")
                    emit_rsqrt(nc, sm_pool, rinv, nrm2[:], [N, 1], "f",
                               n_newton=2)
                    red = sm_pool.tile([N, K], F32, tag="red")
                    nc.vector.tensor_reduce(
                        red[:], om[:].rearrange("p (g k) -> p k g", k=K),
                        axis=Axis.X, op=Alu.add)
                    nc.vector.tensor_scalar_mul(
                        out_stage[:, K * e:K * (e + 1)], red[:], rinv[:])
                    continue

                # --- accT[m, n] = sum_d W[d,m] CX^T[d,n] ; S = mask * accT ---
                pat = psm[:, 1]
                for mc in range(2):
                    for dc in range(2):
                        nc.tensor.matmul(
                            pat[:, mc, :],
                            Wt[:, dc, 128 * mc:128 * (mc + 1)],
                            cx[:, dc, :],
                            start=(dc == 0), stop=(dc == 1),
                            skip_group_check=True)
                S = sm_pool.tile([128, 2, N], F32R, tag="S")
                nc.vector.tensor_mul(S[:], pat[:], mask[:])
                S2 = sm_pool.tile([128, 2, N], F32R, tag="S2")
                nc.vector.tensor_mul(S2[:], S[:], S[:])
                # nrm2 row [1, n] = sum_m S[m,n]^2
                pn = psm[0:1, 3, 0]
                for mc in range(2):
                    nc.tensor.matmul(pn[:], onesc[:], S2[:, mc, :],
                                     start=(mc == 0), stop=(mc == 1))
                r_row = sm_pool.tile([1, N], F32, tag="r_row")
                emit_rsqrt(nc, sm_pool, r_row, pn[:], [1, N], "r", n_newton=1)
                rb = sm_pool.tile([128, N], F32, tag="rb")
                nc.gpsimd.partition_broadcast(rb[:], r_row[:])
                # --- WS[d, n] = sum_m W[d,m] S[m,n], scaled by rinv[n] ---
                pws = psm[:, 2]
                for dc in range(2):
                    for mc in range(2):
                        nc.tensor.matmul(
                            pws[:, dc, :],
                            WTt[:, mc, 128 * dc:128 * (dc + 1)],
                            S[:, mc, :],
                            start=(mc == 0), stop=(mc == 1),
                            skip_group_check=True)
                ws = sm_pool.tile([128, 2, N], BF16, tag="ws_sb")
                for dc in range(2):
                    nc.vector.tensor_mul(ws[:, dc, :], pws[:, dc, :], rb[:])
                # --- b[i, (t,n)] = sum_d x[i,d] WS[d,n] ---
                pb = ps_b.tile([128, T, N], F32, tag="b")
                for t in range(T):
                    for dc in range(2):
                        nc.tensor.matmul(
                            pb[:, t, :],
                            xT_sb[e][:, dc, 128 * t:128 * (t + 1)],
                            ws[:, dc, :],
                            start=(dc == 0), stop=(dc == 1),
                            skip_group_check=True)
                # --- softmax over n ---
                e_sb = ec_pool.tile([128, T, N], BF16, tag="e")
                nc.scalar.activation(e_sb[:], pb[:], Act.Exp)
                s_sum = sm_pool.tile([128, T], F32, tag="s_sum")
                nc.vector.tensor_reduce(s_sum[:], e_sb[:], axis=Axis.X,
                                        op=Alu.add)
                s_r = sm_pool.tile([128, T], F32, tag="s_r")
                nc.vector.reciprocal(s_r[:], s_sum[:])
                c_sb = ec_pool.tile([128, T, N], BF16, tag="c")
                nc.vector.tensor_mul(c_sb[:], e_sb[:],
                                     s_r[:].to_broadcast([128, T, N]))

        # ---- store outputs ----
        nc.sync.dma_start(out_d.ap().rearrange("e n k -> n e k"),
                          out_stage[:].rearrange("p (e k) -> p e k", k=K))

    nc.compile()
    return nc


_NC_CACHE = {}


def _get_nc(n_ex=4, routings=3):
    key = (n_ex, routings)
    if key not in _NC_CACHE:
        _NC_CACHE[key] = build_kernel(*key)
    return _NC_CACHE[key]


def make_const_inputs(W):
    W0 = np.asarray(W[0], dtype=np.float32)  # [D, M]
    Wt = np.ascontiguousarray(
        W0.reshape(2, 128, M).transpose(1, 0, 2))        # [128, 2, M]
    WTt = np.ascontiguousarray(
        W0.T.reshape(2, 128, D).transpose(1, 0, 2))      # [128, 2, D]
    mask = np.zeros((128, 2, N), dtype=BF)
    for p in range(128):
        for mc in range(2):
            mask[p, mc, (mc * 128 + p) // K] = 1.0
    ones16 = np.full((128, N), 1.0 / N, dtype=BF)
    onesc = np.ones((128, 1), dtype=np.float32)
    bmask = np.zeros((N, M), dtype=np.float32)
    for n in range(N):
        bmask[n, n * K:(n + 1) * K] = 1.0
    return Wt, WTt, mask, ones16, onesc, bmask


def kernel(x, W, num_capsule=None, dim_capsule=None, routings=None, **_):
    x = np.asarray(x, dtype=np.float32)
    W = np.asarray(W, dtype=np.float32)
    assert x.shape == (B, IN, D), x.shape

    nc = _get_nc()
    Wt, WTt, mask, ones16, onesc, bmask = make_const_inputs(W)

    n_per = B // N_CORES
    in_maps = []
    for c in range(N_CORES):
        xs = x[c * n_per:(c + 1) * n_per]                 # [4, 4096, 256]
        xq = np.ascontiguousarray(
            xs.reshape(n_per, T, 128, D).transpose(0, 2, 1, 3)).astype(BF)
        xTq = np.ascontiguousarray(
            xs.transpose(0, 2, 1).reshape(n_per, 2, 128, IN)
            .transpose(0, 2, 1, 3)).astype(BF)
        in_maps.append({"xq": xq, "xTq": xTq, "Wt": Wt, "WTt": WTt,
                        "maskmn": mask, "ones16": ones16, "onesc": onesc,
                        "bmask": bmask})

    res = run_bass_kernel_spmd(nc, in_maps, core_ids=list(range(N_CORES)))
    out = np.concatenate([r["out"] for r in res.results], axis=0)
    return out.astype(np.float32)
